# revision 10
# baseline (speedup 1.0000x reference)
"""BatchNorm2d with cubic-spline-interpolated per-channel statistics.

out = x * scale + shift, where scale/shift come from natural-cubic-spline
evaluation of four [T, C] parameter tracks (mean/var/weight/bias) at a
scalar time t:
    scale = weight(t) / sqrt(var(t) + eps)
    shift = bias(t) - mean(t) * scale

Sharding: data-parallel over batch across 8 NeuronCores (4 images each);
the tiny spline parameter tensors are replicated on every core.

Spline evaluation at a fixed scalar t is linear in the knot values, so the
host reduces the time grid to a 10-element basis-weight vector w (by pushing
the identity basis through the spline construction); each core contracts the
replicated parameter tracks with w on-device and streams x through a fused
per-channel affine.

Streaming layout: x/y stream through the device in fp16 (x is host-cast, y
host-upcast; the affine itself is computed against f32 scale/shift, so the
only error is fp16 rounding of x and y, ~6e-4 rel vs the 2e-2 gate) which
halves HBM traffic and is worth 2.2x. x is viewed as [4096, 784] (each
[C=256, 3136] image block row split into 4 contiguous 784-pixel pieces), so
every [128, 784] chunk is one fully contiguous 196KiB DRAM run and each
SBUF partition still holds exactly one channel (channel = vrow // 4). The
per-chunk scale/shift then live in a [128, 8] table indexed by chunk % 8.
DMAs are braided over three descriptor-generation rings (SP + ACT HWDGE and
gpsimd SWDGE) so no single ring's per-DMA overhead becomes the serial
bottleneck. HW-measured 33.6us/stream vs 73.3us for the same shape in f32
(98KiB fp16 chunks regress to 64.5us - keep descriptors at 1568B+).
"""

import numpy as np

B, C, H, W = 32, 256, 56, 56
T = 10
EPS = 1e-5
N_CORES = 8
BPC = B // N_CORES        # batch images per core

# streaming config (winner of HW sweeps)
LAYOUT = "contig"         # 'contig' | 'grouped'
HC = 1568                 # chunk free-dim (per-partition contiguous run)
BUFS = 6
RINGS = 3
IO_DTYPE = np.float16     # x/y stream dtype on device; fp16 halves HBM
                          # traffic (x host-cast, y host-upcast); rounding
                          # adds ~1e-3 rel err vs the 2e-2 gate

S = (H * W) // HC         # pieces per image row
NCOLS = 2 * S             # distinct scale-table columns
ROWS = BPC * C * S        # dram-tensor first dim per core (test.py compat)
HWSZ = HC                 # dram-tensor second dim per core (test.py compat)
NCHUNK = ROWS // 128      # chunks per core

_CACHE = {}


def _spline_basis_weights(times: np.ndarray, t: float) -> np.ndarray:
    """Natural cubic spline eval at t as a linear functional on the knot
    values: eval(times, y, t) == w @ y. Computed by running the spline
    construction on the identity basis (float64 for stability)."""
    times = times.astype(np.float64)
    n = times.shape[0]
    eye = np.eye(n)
    h = np.diff(times)                                   # [n-1]
    slopes = (eye[1:] - eye[:-1]) / h[:, None]           # [n-1, n]
    rhs = 6.0 * (slopes[1:] - slopes[:-1])               # [n-2, n]
    A = (np.diag(2.0 * (h[:-1] + h[1:]))
         + np.diag(h[1:-1], 1)
         + np.diag(h[1:-1], -1))                         # [n-2, n-2]
    m_int = np.linalg.solve(A, rhs)                      # [n-2, n]
    m = np.concatenate([np.zeros((1, n)), m_int, np.zeros((1, n))], axis=0)
    a = eye[:-1]
    b = slopes - h[:, None] * (2.0 * m[:-1] + m[1:]) / 6.0
    c = m[:-1] / 2.0
    d = (m[1:] - m[:-1]) / (6.0 * h[:, None])
    idx = int(np.clip(np.searchsorted(times, t, side="right") - 1, 0, n - 2))
    u = t - times[idx]
    return a[idx] + u * (b[idx] + u * (c[idx] + u * d[idx]))  # [n]


def _grouped_ap(tens, g):
    # rows r = k*128 + p with k = g + NCOLS*j  ->  decompose r = (j g p).
    # Dim order [p, j, c] to match the SBUF tile's [p, (j c)] layout.
    return tens[:, :].rearrange(
        "(j g p) c -> g p j c", j=4, g=NCOLS, p=128
    )[g]


def _build_nc(reps: int = 1):
    # reps>1 re-streams x->y that many times (idempotent); used only by the
    # test harness to measure marginal per-stream HW time.
    import concourse.bacc as bacc
    import concourse.mybir as mybir
    import concourse.tile as tile

    f32 = mybir.dt.float32
    dt_io = mybir.dt.float16 if IO_DTYPE == np.float16 else f32
    nc = bacc.Bacc("TRN2", target_bir_lowering=False, debug=False)

    x = nc.dram_tensor("x", [ROWS, HC], dt_io, kind="ExternalInput")
    pcols = 4 * NCOLS * T
    # pt[p, (param*NCOLS + m)*T + k] = param[k, m*(128//S) + p//S]
    pt = nc.dram_tensor("pt", [128, pcols], f32, kind="ExternalInput")
    # wb[p, j*T + k] = w[k]  (spline basis weights, replicated)
    wb = nc.dram_tensor("wb", [128, pcols], f32, kind="ExternalInput")
    y = nc.dram_tensor("y", [ROWS, HC], dt_io, kind="ExternalOutput")

    with tile.TileContext(nc) as tc:
        with (
            tc.tile_pool(name="stats", bufs=1) as sp,
            tc.tile_pool(name="io", bufs=BUFS) as io,
        ):
            # --- per-channel spline stats: contract params with w ---
            pt_t = sp.tile([128, pcols], f32)
            nc.scalar.dma_start(pt_t[:], pt[:, :])
            wb_t = sp.tile([128, pcols], f32)
            nc.scalar.dma_start(wb_t[:], wb[:, :])
            prod = sp.tile([128, pcols], f32)
            nc.vector.tensor_mul(prod[:], pt_t[:], wb_t[:])
            # stats cols: mean[0:n] var[n:2n] wgt[2n:3n] bias[3n:4n]
            n = NCOLS
            stats = sp.tile([128, 4 * n], f32)
            nc.vector.reduce_sum(
                stats[:],
                prod[:].rearrange("p (j k) -> p j k", k=T),
                axis=mybir.AxisListType.X,
            )
            eps_t = sp.tile([128, 1], f32)
            nc.vector.memset(eps_t[:], EPS)
            std = sp.tile([128, n], f32)
            nc.scalar.activation(
                std[:], stats[:, n:2 * n], mybir.ActivationFunctionType.Sqrt,
                bias=eps_t[:],
            )
            inv = sp.tile([128, n], f32)
            nc.vector.reciprocal(inv[:], std[:])
            scl = sp.tile([128, n], f32)
            nc.vector.tensor_mul(scl[:], stats[:, 2 * n:3 * n], inv[:])
            tmp = sp.tile([128, n], f32)
            nc.vector.tensor_mul(tmp[:], stats[:, 0:n], scl[:])
            sh = sp.tile([128, n], f32)
            nc.vector.tensor_sub(sh[:], stats[:, 3 * n:4 * n], tmp[:])

            # --- stream x through the per-channel affine ---
            # 3-ring braid, store ring leading the load ring by 2 chunks
            # (HW-measured ~1.2us/stream faster than the +1 phase).
            if RINGS == 3:
                ring_pairs = [(nc.sync, nc.gpsimd), (nc.scalar, nc.sync),
                              (nc.gpsimd, nc.scalar)]
            else:
                ring_pairs = [(nc.sync, nc.scalar), (nc.scalar, nc.sync)]

            if LAYOUT == "grouped":
                for _ in range(reps):
                    for g in range(NCOLS):
                        le, se = ring_pairs[g % len(ring_pairs)]
                        xt = io.tile([128, 4 * HC], f32, tag="xt")
                        le.dma_start(
                            xt[:].rearrange("p (j c) -> p j c", j=4),
                            _grouped_ap(x, g),
                        )
                        yt = io.tile([128, 4 * HC], f32, tag="yt")
                        nc.vector.tensor_scalar(
                            yt[:], xt[:],
                            scl[:, g:g + 1], sh[:, g:g + 1],
                            op0=mybir.AluOpType.mult, op1=mybir.AluOpType.add,
                        )
                        se.dma_start(
                            _grouped_ap(y, g),
                            yt[:].rearrange("p (j c) -> p j c", j=4),
                        )
            else:
                for _ in range(reps):
                    for k in range(NCHUNK):
                        m = k % NCOLS
                        le, se = ring_pairs[k % len(ring_pairs)]
                        xt = io.tile([128, HC], dt_io, tag="xt")
                        le.dma_start(xt[:], x[k * 128:(k + 1) * 128, :])
                        yt = io.tile([128, HC], dt_io, tag="yt")
                        nc.vector.tensor_scalar(
                            yt[:], xt[:],
                            scl[:, m:m + 1], sh[:, m:m + 1],
                            op0=mybir.AluOpType.mult, op1=mybir.AluOpType.add,
                        )
                        se.dma_start(y[k * 128:(k + 1) * 128, :], yt[:])

    nc.compile()
    return nc


def _get_nc():
    if "nc" not in _CACHE:
        _CACHE["nc"] = _build_nc()
    return _CACHE["nc"]


def make_in_maps(x, means, vars_, bnweights, bnbiases, times, t):
    """Shard x by batch; replicate spline params (channel-partitioned
    layout matching the streaming chunk map) + basis weights per core."""
    w = _spline_basis_weights(np.asarray(times, np.float32),
                              float(np.asarray(t)[0]))
    params = np.stack(
        [np.asarray(p, np.float32) for p in (means, vars_, bnweights, bnbiases)]
    )                                                     # [4, T, 256]
    # channel held by partition p at scale-table column m
    chmap = (np.arange(NCOLS)[None, :] * (128 // S)
             + (np.arange(128)[:, None] // S))            # [128, NCOLS]
    pt = np.empty((128, 4 * NCOLS * T), np.float32)
    for param in range(4):
        for m in range(NCOLS):
            cols = (param * NCOLS + m) * T + np.arange(T)
            pt[:, cols] = params[param, :, chmap[:, m]]
    wb = np.ascontiguousarray(
        np.broadcast_to(w.astype(np.float32), (128, 4 * NCOLS, T))
        .reshape(128, 4 * NCOLS * T)
    )
    x_np = np.ascontiguousarray(np.asarray(x, np.float32)).astype(
        IO_DTYPE
    ).reshape(N_CORES, ROWS, HC)
    return [{"x": x_np[i], "pt": pt, "wb": wb} for i in range(N_CORES)]


def kernel(x, means, vars_, bnweights, bnbiases, times, t):
    from concourse import bass_utils

    nc = _get_nc()
    in_maps = make_in_maps(x, means, vars_, bnweights, bnbiases, times, t)
    res = bass_utils.run_bass_kernel_spmd(nc, in_maps, core_ids=list(range(N_CORES)))
    return np.concatenate(
        [np.asarray(res.results[i]["y"]).astype(np.float32)
         .reshape(BPC, C, H, W) for i in range(N_CORES)],
        axis=0,
    )
